# revision 16
# baseline (speedup 1.0000x reference)
"""Trainium2 Bass kernel for nn_DSEBlock: FEA (multi-scale bilinear edge) +
DoG (difference-of-gaussians depthwise) + 1x1 mixer, data-parallel over batch
on 8 NeuronCores.

Decomposition (validated vs reference to ~1e-6 in fp64):
  y = dec + skip
  per scale s in {.25,.5,.75}:  r_s = (Uh Dh) x (Uw Dw) y ; d_s = y - r_s
  w_edge = (2/3)(max_s|d_s| - min_s|d_s|)
  dog    = n1*G1 - n2*G2, G_i = separable [t,1,t] x [t,1,t] conv (zero pad)
  out    = mixer @ (3y + w_fea*w_edge + dog) + skip

Engine mapping:
  - back half in C-layout [c(128part), h, w]; FEA front in W-layout
    [w(96part), h(planar), c] reached via PE transposes.
  - W-axis resize = 96x96 matrix on PE.  H-axis: down on DVE; the up-lerp,
    the y-subtract are fused into residue-scaled accumulating PE matmuls
    (psum = y - r directly, in h-residue-planar order).
  - |d| on ACT; running max/min on DVE; wedge transposed back on PE with
    w_fea' folded into the PSUM->SBUF copy.
  - DoG separable on DVE with gaussian-2 chain on GPSIMD.  3y folds into the
    mixer as extra accumulating matmuls with weights 3*M.
"""
import functools

import ml_dtypes
import numpy as np

import concourse.bass as bass
import concourse.mybir as mybir
import concourse.tile as tile
from concourse import bacc
from concourse.bass import ts
from concourse.bass_utils import run_bass_kernel_spmd
from concourse.masks import make_identity

F32 = mybir.dt.float32
BF16 = mybir.dt.bfloat16
AL = mybir.AluOpType
AF = mybir.ActivationFunctionType

B, C, H, W = 16, 256, 96, 96
NCORES = 8
BPC = B // NCORES
SCALES = [0.25, 0.5, 0.75]
NS = [24, 48, 72]
HW = H * W


def _sl(start, step, cnt):
    return slice(start, start + step * (cnt - 1) + 1, step)


# ---------------- host-side resize specs ----------------
def _resize_matrix(n_in, n_out):
    R = np.zeros((n_out, n_in), dtype=np.float64)
    scale = n_in / n_out
    for j in range(n_out):
        x = (j + 0.5) * scale - 0.5
        i0 = int(np.floor(x))
        f = x - i0
        R[j, min(max(i0, 0), n_in - 1)] += 1.0 - f
        R[j, min(max(i0 + 1, 0), n_in - 1)] += f
    return R


def _down_ops(s):
    if s == 0.25:
        return [("avg", (0, 1, 24), (1, 4), (2, 4))]
    if s == 0.5:
        return [("avg", (0, 1, 48), (0, 2), (1, 2))]
    assert s == 0.75
    return [
        ("lerp", (r, 3, 24), (i0, 4), (i0 + 1, 4), f)
        for r, (i0, f) in enumerate([(0, 1 / 6), (1, 1 / 2), (2, 5 / 6)])
    ]


def _up_ops(s):
    """Interior lerp runs per residue r=j%4 plus edge-clamp copies.

    returns (runs, copies): runs[r] = (m0, cnt, a0, S, f) covering
    j = 4m+r, m in [m0, m0+cnt); z indices a0 + S*(m-m0) (+1).
    copies: list of (j, src)."""
    ns = int(H * s)
    scale = ns / H
    S = {0.25: 1, 0.5: 2, 0.75: 3}[s]
    groups, copies = {}, []
    for j in range(H):
        x = (j + 0.5) * scale - 0.5
        i0 = int(np.floor(x))
        f = x - i0
        if i0 < 0:
            copies.append((j, 0))
            continue
        if i0 + 1 > ns - 1:
            copies.append((j, ns - 1))
            continue
        groups.setdefault(j % 4, []).append((j // 4, i0, f))
    runs = {}
    for r, items in groups.items():
        items.sort()
        ms = [m for m, _, _ in items]
        assert ms == list(range(ms[0], ms[-1] + 1))
        fs = {round(f, 9) for _, _, f in items}
        assert len(fs) == 1
        runs[r] = (ms[0], len(ms), items[0][1], S, items[0][2])
    return runs, copies


# ---------------- program (input-independent; cached) ----------------
@functools.lru_cache(maxsize=1)
def _build():
    nc = bacc.Bacc("TRN2", target_bir_lowering=False, debug=False)
    dec_d = nc.dram_tensor("dec", [BPC, C, H, W], F32, kind="ExternalInput")
    skip_d = nc.dram_tensor("skip", [BPC, C, H, W], F32, kind="ExternalInput")
    aw_d = nc.dram_tensor("aw", [96, 27, 96], BF16, kind="ExternalInput")
    mw_d = nc.dram_tensor("mw", [128, 4, 128], BF16, kind="ExternalInput")
    coef_d = nc.dram_tensor("coef", [128, 12], F32, kind="ExternalInput")
    out_d = nc.dram_tensor("out", [BPC, C, H, W], F32, kind="ExternalOutput")

    dn_ops = [_down_ops(s) for s in SCALES]
    up_runs = [_up_ops(s)[0] for s in SCALES]
    up_cp = [_up_ops(s)[1] for s in SCALES]
    # clamp slots by planar position p = r*24 + m
    cp_by_p = [
        {(j % 4) * 24 + j // 4: (j, src) for j, src in up_cp[si]} for si in range(3)
    ]

    with tile.TileContext(nc) as tc:
        with (
            tc.tile_pool(name="const", bufs=1) as pconst,
            tc.tile_pool(name="py", bufs=1) as py,
            tc.tile_pool(name="pyw", bufs=1) as pyw,
            tc.tile_pool(name="pmm", bufs=1) as pmm,
            tc.tile_pool(name="pwk", bufs=2) as pwk,
            tc.tile_pool(name="pdl", bufs=1) as pdl,
            tc.tile_pool(name="pwc", bufs=2) as pwc,
            tc.tile_pool(name="pband", bufs=1) as pband,
            tc.tile_pool(name="pstage", bufs=2) as pstage,
            tc.tile_pool(name="pup", bufs=1) as pup,
            tc.tile_pool(name="pchunk", bufs=2) as pchunk,
            tc.tile_pool(name="ps_ytr", bufs=2, space="PSUM") as ps_ytr,
            tc.tile_pool(name="ps_d", bufs=2, space="PSUM") as ps_d,
            tc.tile_pool(name="ps_wed", bufs=2, space="PSUM") as ps_wed,
            tc.tile_pool(name="ps_mix", bufs=2, space="PSUM") as ps_mix,
        ):
            aw_sb = pconst.tile([96, 27, 96], BF16)
            nc.sync.dma_start(out=aw_sb[:], in_=aw_d[:])
            mw_sb = pconst.tile([128, 4, 128], BF16)
            nc.sync.dma_start(out=mw_sb[:], in_=mw_d[:])
            coef_sb = pconst.tile([128, 12], F32)
            nc.sync.dma_start(out=coef_sb[:], in_=coef_d[:])
            ident = pconst.tile([128, 128], BF16)
            make_identity(nc, ident[:])

            def cf(cb, j, psl=slice(0, 128)):
                return coef_sb[psl, cb * 6 + j : cb * 6 + j + 1]

            def emit_y(s, cb):
                csl_d = slice(cb * 128, (cb + 1) * 128)
                y = py.tile([128, H, W], BF16, tag=f"y{cb}", name=f"y_{s}_{cb}")
                for st in range(8):
                    rsl = slice(st * 12, st * 12 + 12)
                    td = pstage.tile([128, 12, W], BF16, tag="std", name=f"td{s}{cb}{st}")
                    nc.gpsimd.dma_start(out=td[:], in_=dec_d[s, csl_d, rsl])
                    tk = pstage.tile([128, 12, W], BF16, tag="stk", name=f"tk{s}{cb}{st}")
                    nc.gpsimd.dma_start(out=tk[:], in_=skip_d[s, csl_d, rsl])
                    nc.vector.tensor_add(out=y[:, rsl, :], in0=td[:], in1=tk[:])
                return y

            def emit_hdown(si, yw, cfs, s, cb, hf):
                hd = pwk.tile([96, H, 64], BF16, tag="wk", name=f"hd{s}{cb}{hf}{si}")
                for op in dn_ops[si]:
                    if op[0] == "avg":
                        (o0, ostep, cnt), (a0, astep), (b0, bstep) = op[1:]
                        nc.vector.tensor_add(
                            out=hd[:, _sl(o0, ostep, cnt), :],
                            in0=yw[:, _sl(a0, astep, cnt), cfs],
                            in1=yw[:, _sl(b0, bstep, cnt), cfs],
                        )
                    else:
                        (o0, ostep, cnt), (a0, astep), (b0, bstep), f = op[1:]
                        dl = pdl.tile([96, 24, 64], BF16, tag="dl")
                        nc.vector.tensor_sub(
                            out=dl[:],
                            in0=yw[:, _sl(b0, bstep, cnt), cfs],
                            in1=yw[:, _sl(a0, astep, cnt), cfs],
                        )
                        nc.vector.scalar_tensor_tensor(
                            out=hd[:, _sl(o0, ostep, cnt), :],
                            in0=dl[:],
                            scalar=float(f),
                            in1=yw[:, _sl(a0, astep, cnt), cfs],
                            op0=AL.mult,
                            op1=AL.add,
                        )
                return hd

            def emit_dbanks(si, hd, yw, cfs, mx, abs_dst):
                for b in range(12):
                    r = (8 * b) // 24
                    mlo = (8 * b) % 24
                    pd = ps_d.tile([96, 8, 64], F32, tag="pd")
                    m0, cnt, a0, S0, f = up_runs[si][r]
                    ilo, ihi = max(mlo, m0), min(mlo + 8, m0 + cnt)
                    # single accumulation group per bank: y first (start),
                    # then all taps accumulate, last one stops.
                    mms = []
                    if ihi > ilo:
                        n = ihi - ilo
                        av = a0 + S0 * (ilo - m0)
                        sl_o = pd[:, ilo - mlo : ihi - mlo, :]
                        mms.append((sl_o, si * 9 + 2 * r, hd[:, _sl(av, S0, n), :]))
                        mms.append((sl_o, si * 9 + 2 * r + 1, hd[:, _sl(av + 1, S0, n), :]))
                    for mm in range(mlo, mlo + 8):
                        pp = r * 24 + mm
                        if pp in cp_by_p[si]:
                            _, src = cp_by_p[si][pp]
                            mms.append((pd[:, mm - mlo, :], si * 9 + 8, hd[:, src, :]))
                    nc.tensor.matmul(
                        pd[:],
                        lhsT=ident[0:96, 0:96],
                        rhs=yw[:, _sl(4 * mlo + r, 4, 8), cfs],
                        start=True, stop=False,
                    )
                    for i, (out_ap, vi, rhs_ap) in enumerate(mms):
                        nc.tensor.matmul(
                            out_ap,
                            lhsT=aw_sb[:, vi, :],
                            rhs=rhs_ap,
                            start=False,
                            stop=(i == len(mms) - 1),
                        )
                    nc.scalar.activation(abs_dst[:, 8 * b : 8 * b + 8, :], pd[:], AF.Abs)

            def emit_front(s, cb, y):
                yw = pyw.tile([96, H, 128], BF16, tag="yw", name=f"yw_{s}_{cb}")
                for hb in range(24):
                    pt = ps_ytr.tile([96, 4, 128], BF16, tag="ptr", name=f"pt{s}{cb}{hb}")
                    for k in range(4):
                        nc.tensor.transpose(pt[:, k, :], y[:, hb * 4 + k, :], ident[:])
                    nc.scalar.copy(yw[:, hb * 4 : hb * 4 + 4, :], pt[:])
                mx = pmm.tile([96, H, 128], BF16, tag="mx", name=f"mx{s}{cb}")
                for hf in range(2):
                    cfs = slice(hf * 64, hf * 64 + 64)
                    mn = pmm.tile([96, H, 64], BF16, tag="mn", name=f"mn{s}{cb}{hf}")
                    for si in range(3):
                        hd = emit_hdown(si, yw, cfs, s, cb, hf)
                        abs_dst = (
                            mn if si == 0
                            else pwk.tile([96, H, 64], BF16, tag="wk", name=f"ab{s}{cb}{hf}{si}")
                        )
                        emit_dbanks(si, hd, yw, cfs, mx, abs_dst)
                        if si == 1:
                            nc.vector.tensor_tensor(
                                out=mx[:, :, cfs], in0=mn[:], in1=abs_dst[:], op=AL.max
                            )
                            nc.vector.tensor_tensor(
                                out=mn[:], in0=mn[:], in1=abs_dst[:], op=AL.min
                            )
                        elif si == 2:
                            nc.vector.tensor_tensor(
                                out=mx[:, :, cfs], in0=mx[:, :, cfs], in1=abs_dst[:], op=AL.max
                            )
                            nc.vector.tensor_tensor(
                                out=mn[:], in0=mn[:], in1=abs_dst[:], op=AL.min
                            )
                    nc.vector.tensor_sub(out=mx[:, :, cfs], in0=mx[:, :, cfs], in1=mn[:])
                wcon = pwc.tile([128, H, W], BF16, tag="wcon", name=f"wc_{s}_{cb}")
                for q in range(24):
                    pw = ps_wed.tile([128, 4, 96], BF16, tag="pwed")
                    for i in range(4):
                        nc.tensor.transpose(pw[:, i, :], mx[:, 4 * q + i, :], ident[0:96, 0:96])
                    p0 = 4 * q
                    r, m = p0 // 24, p0 % 24
                    nc.scalar.activation(
                        wcon[:, _sl(4 * m + r, 4, 4), :], pw[:], AF.Copy, scale=cf(cb, 0)
                    )
                return wcon

            def emit_back(s, cb, y, wcon):
                """wcon := wf'*wedge + (3+a)y + beta*(Nw+Nh)y + gamma*NhNw y."""
                up = pup.tile([128, H, W], BF16, tag="up", name=f"up{s}{cb}")
                nc.scalar.activation(up[:], y[:], AF.Copy, scale=cf(cb, 1))
                for bi in range(12):
                    r0, r1 = bi * 8, bi * 8 + 8
                    ra0, ra1 = max(r0 - 1, 0), min(r1 + 1, H)
                    na = ra1 - ra0
                    at = pband.tile([128, 10, W], BF16, tag="at", name=f"at{s}{cb}{bi}")
                    nc.vector.tensor_add(
                        out=at[:, :na, 1:95],
                        in0=y[:, ra0:ra1, 0:94],
                        in1=y[:, ra0:ra1, 2:96],
                    )
                    nc.vector.tensor_copy(out=at[:, :na, 0], in_=y[:, ra0:ra1, 1])
                    nc.vector.tensor_copy(out=at[:, :na, 95], in_=y[:, ra0:ra1, 94])
                    # ab = Nh y + a  (rows r0..r1)
                    ab = pband.tile([128, 8, W], BF16, tag="ab", name=f"ab{s}{cb}{bi}")
                    g0, g1 = max(r0, 1), min(r1, 95)
                    nc.gpsimd.tensor_tensor(
                        out=ab[:, g0 - r0 : g1 - r0, :],
                        in0=y[:, g0 - 1 : g1 - 1, :],
                        in1=y[:, g0 + 1 : g1 + 1, :],
                        op=AL.add,
                    )
                    if r0 == 0:
                        nc.gpsimd.tensor_copy(out=ab[:, 0, :], in_=y[:, 1, :])
                    if r1 == H:
                        nc.gpsimd.tensor_copy(out=ab[:, 7, :], in_=y[:, 94, :])
                    nc.vector.tensor_add(
                        out=ab[:], in0=ab[:], in1=at[:, r0 - ra0 : r0 - ra0 + 8, :]
                    )
                    # c2 = Nh a (rows r0..r1); band-local indices into at
                    c2 = pband.tile([128, 8, W], BF16, tag="c2", name=f"c2{s}{cb}{bi}")
                    nc.vector.tensor_add(
                        out=c2[:, g0 - r0 : g1 - r0, :],
                        in0=at[:, g0 - 1 - ra0 : g1 - 1 - ra0, :],
                        in1=at[:, g0 + 1 - ra0 : g1 + 1 - ra0, :],
                    )
                    if r0 == 0:
                        nc.vector.tensor_copy(out=c2[:, 0, :], in_=at[:, 1, :])
                    if r1 == H:
                        nc.vector.tensor_copy(out=c2[:, 7, :], in_=at[:, 94 - ra0, :])
                    nc.vector.scalar_tensor_tensor(
                        out=up[:, r0:r1, :], in0=ab[:], scalar=cf(cb, 2),
                        in1=up[:, r0:r1, :], op0=AL.mult, op1=AL.add,
                    )
                    nc.vector.scalar_tensor_tensor(
                        out=up[:, r0:r1, :], in0=c2[:], scalar=cf(cb, 3),
                        in1=up[:, r0:r1, :], op0=AL.mult, op1=AL.add,
                    )
                nc.vector.tensor_add(out=wcon[:], in0=wcon[:], in1=up[:])

            def emit_mix(s, us):
                uf = [t[:].rearrange("c h w -> c (h w)") for t in us]
                for ob in range(2):
                    osl = slice(ob * 128, (ob + 1) * 128)
                    of = out_d[s, osl].rearrange("c h w -> c (h w)")
                    kf = skip_d[s, osl].rearrange("c h w -> c (h w)")
                    for ng in range(18):
                        sk = pchunk.tile([128, 512], F32, tag="sk")
                        nc.sync.dma_start(out=sk[:], in_=kf[:, ts(ng, 512)])
                        pmx = ps_mix.tile([128, 512], F32, tag="pmix")
                        nc.tensor.matmul(
                            pmx[:], lhsT=mw_sb[:, ob, :], rhs=uf[0][:, ts(ng, 512)],
                            start=True, stop=False,
                        )
                        nc.tensor.matmul(
                            pmx[:], lhsT=mw_sb[:, 2 + ob, :], rhs=uf[1][:, ts(ng, 512)],
                            start=False, stop=True,
                        )
                        ot = pchunk.tile([128, 512], F32, tag="ot")
                        nc.vector.tensor_add(out=ot[:], in0=pmx[:], in1=sk[:])
                        nc.sync.dma_start(out=of[:, ts(ng, 512)], in_=ot[:])

            for s in range(BPC):
                us = []
                for cb in range(2):
                    y = emit_y(s, cb)
                    wcon = emit_front(s, cb, y)
                    emit_back(s, cb, y, wcon)
                    us.append(wcon)
                emit_mix(s, us)
    nc.finalize()
    return nc


# ---------------- host entry ----------------
def _consts(w_fea, sigma1, sigma2, mixer_w):
    wf = (w_fea.reshape(C).astype(np.float64)) * (2.0 / 3.0)
    tn = []
    for sg in (sigma1, sigma2):
        sig = 2.0 / (1.0 + np.exp(-sg.reshape(C).astype(np.float64)))
        t = np.exp(-1.0 / (2.0 * sig**2))
        tn.append((t, (1.0 + 2.0 * t) ** -2))
    (t1, n1), (t2, n2) = tn
    c_y = 3.0 + n1 - n2
    c_ab = n1 * t1 - n2 * t2
    c_c2 = n1 * t1**2 - n2 * t2**2
    coef = np.zeros((128, 12), dtype=np.float32)
    for cb in range(2):
        ch = slice(cb * 128, (cb + 1) * 128)
        coef[:, cb * 6 + 0] = wf[ch]
        coef[:, cb * 6 + 1] = c_y[ch]
        coef[:, cb * 6 + 2] = c_ab[ch]
        coef[:, cb * 6 + 3] = c_c2[ch]

    aw = np.zeros((96, 27, 96), dtype=np.float64)
    for si, s in enumerate(SCALES):
        ns = int(H * s)
        A = _resize_matrix(ns, H) @ _resize_matrix(H, ns)
        fold = 0.5 if s in (0.25, 0.5) else 1.0
        Af = fold * A  # (96h', ns-ish) acting along the W axis: [w', w]
        runs, _ = _up_ops(s)
        for r, (m0, cnt, a0, S0, f) in runs.items():
            aw[:, si * 9 + 2 * r, :] = (-(1.0 - f) * Af).T
            aw[:, si * 9 + 2 * r + 1, :] = (-f * Af).T
        aw[:, si * 9 + 8, :] = (-Af).T
    aw = aw.astype(ml_dtypes.bfloat16)

    M = mixer_w.reshape(C, C).astype(np.float64)
    mw = np.zeros((128, 4, 128), dtype=np.float64)
    for kc in range(2):
        for ob in range(2):
            blk = M[ob * 128 : (ob + 1) * 128, kc * 128 : (kc + 1) * 128].T
            mw[:, kc * 2 + ob, :] = blk
    mw = mw.astype(ml_dtypes.bfloat16)
    return aw, mw, coef


def kernel(skip, dec, w_fea, sigma1, sigma2, mixer_w, _trace=[False]):
    skip = np.ascontiguousarray(np.asarray(skip, dtype=np.float32))
    dec = np.ascontiguousarray(np.asarray(dec, dtype=np.float32))
    aw, mw, coef = _consts(
        np.asarray(w_fea), np.asarray(sigma1), np.asarray(sigma2), np.asarray(mixer_w)
    )
    nc = _build()
    in_maps = []
    for i in range(NCORES):
        in_maps.append(
            {
                "dec": dec[BPC * i : BPC * (i + 1)],
                "skip": skip[BPC * i : BPC * (i + 1)],
                "aw": aw,
                "mw": mw,
                "coef": coef,
            }
        )
    res = run_bass_kernel_spmd(nc, in_maps, core_ids=list(range(NCORES)), trace=_trace[0])
    kernel.last_result = res
    return np.concatenate([r["out"] for r in res.results], axis=0)


kernel.last_result = None



# revision 17
# speedup vs baseline: 1.0408x; 1.0408x over previous
"""Trainium2 Bass kernel for nn_DSEBlock: FEA (multi-scale bilinear edge) +
DoG (difference-of-gaussians depthwise) + 1x1 mixer, data-parallel over batch
on 8 NeuronCores.

Decomposition (validated vs reference to ~1e-6 in fp64):
  y = dec + skip
  per scale s in {.25,.5,.75}:  r_s = (Uh Dh) x (Uw Dw) y ; d_s = y - r_s
  w_edge = (2/3)(max_s|d_s| - min_s|d_s|)
  dog    = n1*G1 - n2*G2, G_i = separable [t,1,t] x [t,1,t] conv (zero pad)
  out    = mixer @ (3y + w_fea*w_edge + dog) + skip

Engine mapping:
  - back half in C-layout [c(128part), h, w]; FEA front in W-layout
    [w(96part), h(planar), c] reached via PE transposes.
  - W-axis resize = 96x96 matrix on PE.  H-axis: down on DVE; the up-lerp,
    the y-subtract are fused into residue-scaled accumulating PE matmuls
    (psum = y - r directly, in h-residue-planar order).
  - |d| on ACT; running max/min on DVE; wedge transposed back on PE with
    w_fea' folded into the PSUM->SBUF copy.
  - DoG separable on DVE with gaussian-2 chain on GPSIMD.  3y folds into the
    mixer as extra accumulating matmuls with weights 3*M.
"""
import functools

import ml_dtypes
import numpy as np

import concourse.bass as bass
import concourse.mybir as mybir
import concourse.tile as tile
from concourse import bacc
from concourse.bass import ts
from concourse.bass_utils import run_bass_kernel_spmd
from concourse.masks import make_identity

F32 = mybir.dt.float32
BF16 = mybir.dt.bfloat16
AL = mybir.AluOpType
AF = mybir.ActivationFunctionType

B, C, H, W = 16, 256, 96, 96
NCORES = 8
BPC = B // NCORES
SCALES = [0.25, 0.5, 0.75]
NS = [24, 48, 72]
HW = H * W


def _sl(start, step, cnt):
    return slice(start, start + step * (cnt - 1) + 1, step)


# ---------------- host-side resize specs ----------------
def _resize_matrix(n_in, n_out):
    R = np.zeros((n_out, n_in), dtype=np.float64)
    scale = n_in / n_out
    for j in range(n_out):
        x = (j + 0.5) * scale - 0.5
        i0 = int(np.floor(x))
        f = x - i0
        R[j, min(max(i0, 0), n_in - 1)] += 1.0 - f
        R[j, min(max(i0 + 1, 0), n_in - 1)] += f
    return R


def _down_ops(s):
    if s == 0.25:
        return [("avg", (0, 1, 24), (1, 4), (2, 4))]
    if s == 0.5:
        return [("avg", (0, 1, 48), (0, 2), (1, 2))]
    assert s == 0.75
    return [
        ("lerp", (r, 3, 24), (i0, 4), (i0 + 1, 4), f)
        for r, (i0, f) in enumerate([(0, 1 / 6), (1, 1 / 2), (2, 5 / 6)])
    ]


def _up_ops(s):
    """Interior lerp runs per residue r=j%4 plus edge-clamp copies.

    returns (runs, copies): runs[r] = (m0, cnt, a0, S, f) covering
    j = 4m+r, m in [m0, m0+cnt); z indices a0 + S*(m-m0) (+1).
    copies: list of (j, src)."""
    ns = int(H * s)
    scale = ns / H
    S = {0.25: 1, 0.5: 2, 0.75: 3}[s]
    groups, copies = {}, []
    for j in range(H):
        x = (j + 0.5) * scale - 0.5
        i0 = int(np.floor(x))
        f = x - i0
        if i0 < 0:
            copies.append((j, 0))
            continue
        if i0 + 1 > ns - 1:
            copies.append((j, ns - 1))
            continue
        groups.setdefault(j % 4, []).append((j // 4, i0, f))
    runs = {}
    for r, items in groups.items():
        items.sort()
        ms = [m for m, _, _ in items]
        assert ms == list(range(ms[0], ms[-1] + 1))
        fs = {round(f, 9) for _, _, f in items}
        assert len(fs) == 1
        runs[r] = (ms[0], len(ms), items[0][1], S, items[0][2])
    return runs, copies


# ---------------- program (input-independent; cached) ----------------
@functools.lru_cache(maxsize=1)
def _build():
    nc = bacc.Bacc("TRN2", target_bir_lowering=False, debug=False)
    dec_d = nc.dram_tensor("dec", [BPC, C, H, W], F32, kind="ExternalInput")
    skip_d = nc.dram_tensor("skip", [BPC, C, H, W], F32, kind="ExternalInput")
    aw_d = nc.dram_tensor("aw", [96, 27, 96], BF16, kind="ExternalInput")
    mw_d = nc.dram_tensor("mw", [128, 4, 128], BF16, kind="ExternalInput")
    coef_d = nc.dram_tensor("coef", [128, 12], F32, kind="ExternalInput")
    out_d = nc.dram_tensor("out", [BPC, C, H, W], F32, kind="ExternalOutput")

    dn_ops = [_down_ops(s) for s in SCALES]
    up_runs = [_up_ops(s)[0] for s in SCALES]
    up_cp = [_up_ops(s)[1] for s in SCALES]
    # clamp slots by planar position p = r*24 + m
    cp_by_p = [
        {(j % 4) * 24 + j // 4: (j, src) for j, src in up_cp[si]} for si in range(3)
    ]

    with tile.TileContext(nc) as tc:
        with (
            tc.tile_pool(name="const", bufs=1) as pconst,
            tc.tile_pool(name="py", bufs=1) as py,
            tc.tile_pool(name="pyw", bufs=1) as pyw,
            tc.tile_pool(name="pmm", bufs=1) as pmm,
            tc.tile_pool(name="pwk", bufs=2) as pwk,
            tc.tile_pool(name="pdl", bufs=1) as pdl,
            tc.tile_pool(name="pwc", bufs=2) as pwc,
            tc.tile_pool(name="pband", bufs=1) as pband,
            tc.tile_pool(name="pstage", bufs=2) as pstage,
            tc.tile_pool(name="pup", bufs=1) as pup,
            tc.tile_pool(name="pchunk", bufs=2) as pchunk,
            tc.tile_pool(name="ps_ytr", bufs=2, space="PSUM") as ps_ytr,
            tc.tile_pool(name="ps_d", bufs=2, space="PSUM") as ps_d,
            tc.tile_pool(name="ps_wed", bufs=2, space="PSUM") as ps_wed,
            tc.tile_pool(name="ps_mix", bufs=2, space="PSUM") as ps_mix,
        ):
            aw_sb = pconst.tile([96, 27, 96], BF16)
            nc.sync.dma_start(out=aw_sb[:], in_=aw_d[:])
            mw_sb = pconst.tile([128, 4, 128], BF16)
            nc.sync.dma_start(out=mw_sb[:], in_=mw_d[:])
            coef_sb = pconst.tile([128, 12], F32)
            nc.sync.dma_start(out=coef_sb[:], in_=coef_d[:])
            ident = pconst.tile([128, 128], BF16)
            make_identity(nc, ident[:])

            def cf(cb, j, psl=slice(0, 128)):
                return coef_sb[psl, cb * 6 + j : cb * 6 + j + 1]

            def emit_y(s, cb):
                csl_d = slice(cb * 128, (cb + 1) * 128)
                y = py.tile([128, H, W], BF16, tag=f"y{cb}", name=f"y_{s}_{cb}")
                for st in range(8):
                    rsl = slice(st * 12, st * 12 + 12)
                    td = pstage.tile([128, 12, W], BF16, tag="std", name=f"td{s}{cb}{st}")
                    nc.gpsimd.dma_start(out=td[:], in_=dec_d[s, csl_d, rsl])
                    tk = pstage.tile([128, 12, W], BF16, tag="stk", name=f"tk{s}{cb}{st}")
                    nc.gpsimd.dma_start(out=tk[:], in_=skip_d[s, csl_d, rsl])
                    nc.vector.tensor_add(out=y[:, rsl, :], in0=td[:], in1=tk[:])
                return y

            def emit_hdown(si, yw, cfs, s, cb, hf):
                hd = pwk.tile([96, H, 64], BF16, tag="wk", name=f"hd{s}{cb}{hf}{si}")
                for op in dn_ops[si]:
                    if op[0] == "avg":
                        (o0, ostep, cnt), (a0, astep), (b0, bstep) = op[1:]
                        nc.vector.tensor_add(
                            out=hd[:, _sl(o0, ostep, cnt), :],
                            in0=yw[:, _sl(a0, astep, cnt), cfs],
                            in1=yw[:, _sl(b0, bstep, cnt), cfs],
                        )
                    else:
                        (o0, ostep, cnt), (a0, astep), (b0, bstep), f = op[1:]
                        dl = pdl.tile([96, 24, 64], BF16, tag="dl")
                        nc.vector.tensor_sub(
                            out=dl[:],
                            in0=yw[:, _sl(b0, bstep, cnt), cfs],
                            in1=yw[:, _sl(a0, astep, cnt), cfs],
                        )
                        nc.vector.scalar_tensor_tensor(
                            out=hd[:, _sl(o0, ostep, cnt), :],
                            in0=dl[:],
                            scalar=float(f),
                            in1=yw[:, _sl(a0, astep, cnt), cfs],
                            op0=AL.mult,
                            op1=AL.add,
                        )
                return hd

            def emit_dbanks(si, hd, yw, cfs, mx, abs_dst):
                for b in range(12):
                    r = (8 * b) // 24
                    mlo = (8 * b) % 24
                    pd = ps_d.tile([96, 8, 64], F32, tag="pd")
                    m0, cnt, a0, S0, f = up_runs[si][r]
                    ilo, ihi = max(mlo, m0), min(mlo + 8, m0 + cnt)
                    # single accumulation group per bank: y first (start),
                    # then all taps accumulate, last one stops.
                    mms = []
                    if ihi > ilo:
                        n = ihi - ilo
                        av = a0 + S0 * (ilo - m0)
                        sl_o = pd[:, ilo - mlo : ihi - mlo, :]
                        mms.append((sl_o, si * 9 + 2 * r, hd[:, _sl(av, S0, n), :]))
                        mms.append((sl_o, si * 9 + 2 * r + 1, hd[:, _sl(av + 1, S0, n), :]))
                    for mm in range(mlo, mlo + 8):
                        pp = r * 24 + mm
                        if pp in cp_by_p[si]:
                            _, src = cp_by_p[si][pp]
                            mms.append((pd[:, mm - mlo, :], si * 9 + 8, hd[:, src, :]))
                    nc.tensor.matmul(
                        pd[:],
                        lhsT=ident[0:96, 0:96],
                        rhs=yw[:, _sl(4 * mlo + r, 4, 8), cfs],
                        start=True, stop=False,
                    )
                    for i, (out_ap, vi, rhs_ap) in enumerate(mms):
                        nc.tensor.matmul(
                            out_ap,
                            lhsT=aw_sb[:, vi, :],
                            rhs=rhs_ap,
                            start=False,
                            stop=(i == len(mms) - 1),
                        )
                    if abs_dst is None:
                        nc.scalar.activation(mx[:, 8 * b : 8 * b + 8, cfs], pd[:], AF.Abs)
                    else:
                        nc.scalar.activation(abs_dst[:, 8 * b : 8 * b + 8, :], pd[:], AF.Abs)

            def emit_front(s, cb, y):
                yw = pyw.tile([96, H, 128], BF16, tag="yw", name=f"yw_{s}_{cb}")
                for hb in range(24):
                    pt = ps_ytr.tile([96, 4, 128], BF16, tag="ptr", name=f"pt{s}{cb}{hb}")
                    for k in range(4):
                        nc.tensor.transpose(pt[:, k, :], y[:, hb * 4 + k, :], ident[:])
                    nc.scalar.copy(yw[:, hb * 4 : hb * 4 + 4, :], pt[:])
                mx = pmm.tile([96, H, 128], BF16, tag="mx", name=f"mx{s}{cb}")
                for hf in range(2):
                    cfs = slice(hf * 64, hf * 64 + 64)
                    mn = pmm.tile([96, H, 64], BF16, tag="mn", name=f"mn{s}{cb}{hf}")
                    for si in range(3):
                        hd = emit_hdown(si, yw, cfs, s, cb, hf)
                        abs_dst = (
                            None if si == 0
                            else pwk.tile([96, H, 64], BF16, tag="wk", name=f"ab{s}{cb}{hf}{si}")
                        )
                        emit_dbanks(si, hd, yw, cfs, mx, abs_dst)
                        if si == 0:
                            nc.vector.tensor_copy(out=mn[:], in_=mx[:, :, cfs])
                        else:
                            nc.vector.tensor_tensor(
                                out=mx[:, :, cfs], in0=mx[:, :, cfs], in1=abs_dst[:], op=AL.max
                            )
                            nc.vector.tensor_tensor(
                                out=mn[:], in0=mn[:], in1=abs_dst[:], op=AL.min
                            )
                    nc.vector.tensor_sub(out=mx[:, :, cfs], in0=mx[:, :, cfs], in1=mn[:])
                wcon = pwc.tile([128, H, W], BF16, tag="wcon", name=f"wc_{s}_{cb}")
                for q in range(24):
                    pw = ps_wed.tile([128, 4, 96], BF16, tag="pwed")
                    for i in range(4):
                        nc.tensor.transpose(pw[:, i, :], mx[:, 4 * q + i, :], ident[0:96, 0:96])
                    p0 = 4 * q
                    r, m = p0 // 24, p0 % 24
                    nc.scalar.activation(
                        wcon[:, _sl(4 * m + r, 4, 4), :], pw[:], AF.Copy, scale=cf(cb, 0)
                    )
                return wcon

            def emit_back(s, cb, y, wcon):
                """wcon := wf'*wedge + (3+a)y + beta*(Nw+Nh)y + gamma*NhNw y."""
                up = pup.tile([128, H, W], BF16, tag="up", name=f"up{s}{cb}")
                nc.scalar.activation(up[:], y[:], AF.Copy, scale=cf(cb, 1))
                for bi in range(12):
                    r0, r1 = bi * 8, bi * 8 + 8
                    ra0, ra1 = max(r0 - 1, 0), min(r1 + 1, H)
                    na = ra1 - ra0
                    at = pband.tile([128, 10, W], BF16, tag="at", name=f"at{s}{cb}{bi}")
                    nc.vector.tensor_add(
                        out=at[:, :na, 1:95],
                        in0=y[:, ra0:ra1, 0:94],
                        in1=y[:, ra0:ra1, 2:96],
                    )
                    nc.vector.tensor_copy(out=at[:, :na, 0], in_=y[:, ra0:ra1, 1])
                    nc.vector.tensor_copy(out=at[:, :na, 95], in_=y[:, ra0:ra1, 94])
                    # ab = Nh y + a  (rows r0..r1)
                    ab = pband.tile([128, 8, W], BF16, tag="ab", name=f"ab{s}{cb}{bi}")
                    g0, g1 = max(r0, 1), min(r1, 95)
                    nc.gpsimd.tensor_tensor(
                        out=ab[:, g0 - r0 : g1 - r0, :],
                        in0=y[:, g0 - 1 : g1 - 1, :],
                        in1=y[:, g0 + 1 : g1 + 1, :],
                        op=AL.add,
                    )
                    if r0 == 0:
                        nc.gpsimd.tensor_copy(out=ab[:, 0, :], in_=y[:, 1, :])
                    if r1 == H:
                        nc.gpsimd.tensor_copy(out=ab[:, 7, :], in_=y[:, 94, :])
                    nc.vector.tensor_add(
                        out=ab[:], in0=ab[:], in1=at[:, r0 - ra0 : r0 - ra0 + 8, :]
                    )
                    # c2 = Nh a (rows r0..r1); band-local indices into at
                    c2 = pband.tile([128, 8, W], BF16, tag="c2", name=f"c2{s}{cb}{bi}")
                    nc.vector.tensor_add(
                        out=c2[:, g0 - r0 : g1 - r0, :],
                        in0=at[:, g0 - 1 - ra0 : g1 - 1 - ra0, :],
                        in1=at[:, g0 + 1 - ra0 : g1 + 1 - ra0, :],
                    )
                    if r0 == 0:
                        nc.vector.tensor_copy(out=c2[:, 0, :], in_=at[:, 1, :])
                    if r1 == H:
                        nc.vector.tensor_copy(out=c2[:, 7, :], in_=at[:, 94 - ra0, :])
                    nc.vector.scalar_tensor_tensor(
                        out=up[:, r0:r1, :], in0=ab[:], scalar=cf(cb, 2),
                        in1=up[:, r0:r1, :], op0=AL.mult, op1=AL.add,
                    )
                    nc.vector.scalar_tensor_tensor(
                        out=up[:, r0:r1, :], in0=c2[:], scalar=cf(cb, 3),
                        in1=up[:, r0:r1, :], op0=AL.mult, op1=AL.add,
                    )
                nc.vector.tensor_add(out=wcon[:], in0=wcon[:], in1=up[:])

            def emit_mix(s, us):
                uf = [t[:].rearrange("c h w -> c (h w)") for t in us]
                for ob in range(2):
                    osl = slice(ob * 128, (ob + 1) * 128)
                    of = out_d[s, osl].rearrange("c h w -> c (h w)")
                    kf = skip_d[s, osl].rearrange("c h w -> c (h w)")
                    for ng in range(18):
                        sk = pchunk.tile([128, 512], F32, tag="sk")
                        nc.sync.dma_start(out=sk[:], in_=kf[:, ts(ng, 512)])
                        pmx = ps_mix.tile([128, 512], F32, tag="pmix")
                        nc.tensor.matmul(
                            pmx[:], lhsT=mw_sb[:, ob, :], rhs=uf[0][:, ts(ng, 512)],
                            start=True, stop=False,
                        )
                        nc.tensor.matmul(
                            pmx[:], lhsT=mw_sb[:, 2 + ob, :], rhs=uf[1][:, ts(ng, 512)],
                            start=False, stop=True,
                        )
                        ot = pchunk.tile([128, 512], F32, tag="ot")
                        nc.vector.tensor_add(out=ot[:], in0=pmx[:], in1=sk[:])
                        nc.sync.dma_start(out=of[:, ts(ng, 512)], in_=ot[:])

            for s in range(BPC):
                us = []
                for cb in range(2):
                    y = emit_y(s, cb)
                    wcon = emit_front(s, cb, y)
                    emit_back(s, cb, y, wcon)
                    us.append(wcon)
                emit_mix(s, us)
    nc.finalize()
    return nc


# ---------------- host entry ----------------
def _consts(w_fea, sigma1, sigma2, mixer_w):
    wf = (w_fea.reshape(C).astype(np.float64)) * (2.0 / 3.0)
    tn = []
    for sg in (sigma1, sigma2):
        sig = 2.0 / (1.0 + np.exp(-sg.reshape(C).astype(np.float64)))
        t = np.exp(-1.0 / (2.0 * sig**2))
        tn.append((t, (1.0 + 2.0 * t) ** -2))
    (t1, n1), (t2, n2) = tn
    c_y = 3.0 + n1 - n2
    c_ab = n1 * t1 - n2 * t2
    c_c2 = n1 * t1**2 - n2 * t2**2
    coef = np.zeros((128, 12), dtype=np.float32)
    for cb in range(2):
        ch = slice(cb * 128, (cb + 1) * 128)
        coef[:, cb * 6 + 0] = wf[ch]
        coef[:, cb * 6 + 1] = c_y[ch]
        coef[:, cb * 6 + 2] = c_ab[ch]
        coef[:, cb * 6 + 3] = c_c2[ch]

    aw = np.zeros((96, 27, 96), dtype=np.float64)
    for si, s in enumerate(SCALES):
        ns = int(H * s)
        A = _resize_matrix(ns, H) @ _resize_matrix(H, ns)
        fold = 0.5 if s in (0.25, 0.5) else 1.0
        Af = fold * A  # (96h', ns-ish) acting along the W axis: [w', w]
        runs, _ = _up_ops(s)
        for r, (m0, cnt, a0, S0, f) in runs.items():
            aw[:, si * 9 + 2 * r, :] = (-(1.0 - f) * Af).T
            aw[:, si * 9 + 2 * r + 1, :] = (-f * Af).T
        aw[:, si * 9 + 8, :] = (-Af).T
    aw = aw.astype(ml_dtypes.bfloat16)

    M = mixer_w.reshape(C, C).astype(np.float64)
    mw = np.zeros((128, 4, 128), dtype=np.float64)
    for kc in range(2):
        for ob in range(2):
            blk = M[ob * 128 : (ob + 1) * 128, kc * 128 : (kc + 1) * 128].T
            mw[:, kc * 2 + ob, :] = blk
    mw = mw.astype(ml_dtypes.bfloat16)
    return aw, mw, coef


def kernel(skip, dec, w_fea, sigma1, sigma2, mixer_w, _trace=[False]):
    skip = np.ascontiguousarray(np.asarray(skip, dtype=np.float32))
    dec = np.ascontiguousarray(np.asarray(dec, dtype=np.float32))
    aw, mw, coef = _consts(
        np.asarray(w_fea), np.asarray(sigma1), np.asarray(sigma2), np.asarray(mixer_w)
    )
    nc = _build()
    in_maps = []
    for i in range(NCORES):
        in_maps.append(
            {
                "dec": dec[BPC * i : BPC * (i + 1)],
                "skip": skip[BPC * i : BPC * (i + 1)],
                "aw": aw,
                "mw": mw,
                "coef": coef,
            }
        )
    res = run_bass_kernel_spmd(nc, in_maps, core_ids=list(range(NCORES)), trace=_trace[0])
    kernel.last_result = res
    return np.concatenate([r["out"] for r in res.results], axis=0)


kernel.last_result = None

